# revision 11
# baseline (speedup 1.0000x reference)
"""BilinearInteraction Trainium2 kernel (8 NeuronCores, batch-sharded).

out[b, p=(i,j), d] = x[b, i, d] * (x @ W)[b, j, d]  for the 496 upper-tri
pairs of F=32 fields; x [4096, 32, 64] f32, W [64, 64] f32.

The f32 version of this kernel is pinned at the HBM roofline: 65 MB of
stores per core at the ~358 GB/s HBM-per-NC limit is ~190 us. This
version runs the whole device pipeline in fp16 (intermediate rounding
~1e-3 relative, well inside the 2e-2 gate) and upcasts to f32 on the
host: stores halve to 32.5 MB/core and the DVE multiply runs in the
2x_1P 16-bit perf mode, moving the roofline to ~95-100 us.

Per core: 512 batch rows, processed as 4 tiles of 128 (batch on SBUF
partitions). Per tile, in descending field order so the first-processed
output chunk only needs the tail of vid:
  - vid = x @ W via PE pair-block transposes ([128,128] f-pair blocks
    -> PSUM) + matmuls against a host-provided block-diag [[W,0],[0,W]]
    (two fields per instruction), grouped 4-to-a-PSUM-bank so ACT moves
    PSUM->SBUF in few fat copies (casting f32 PSUM -> fp16 SBUF).
  - pairwise Hadamard on DVE: one tensor_mul per field i covering all
    j>i at once, broadcasting x[:,i,:] over the j axis with a stride-0
    access pattern; innermost dim is 64 contiguous fp16 so the 2x_1P
    packed mode engages.
  - output staged in SBUF in 4 block-aligned chunks (~16 KB/partition
    lines), each DMA'd as one ~2 MB contiguous-per-partition store on
    the sync HWDGE ring; inputs ride the scalar-engine ring so they
    never queue behind output stores.
Ramp: tile 0's x loads high-fields-first (the first-processed chunk
only reads fields >=16) and the first chunk's store is split in three,
so the output stream starts early.
"""

import sys

if "/opt/trn_rl_repo" not in sys.path:
    sys.path.insert(0, "/opt/trn_rl_repo")

import numpy as np

import concourse.bass as bass
import concourse.mybir as mybir
import concourse.tile as tile
from concourse import bacc
from concourse.bass_utils import run_bass_kernel_spmd

B, F, D = 4096, 32, 64
P = F * (F - 1) // 2  # 496
NCORES = 8
BSH = B // NCORES  # 512 batch rows per core
BT = 128  # batch tile (SBUF partitions)
NTILES = BSH // BT  # 4

f32 = mybir.dt.float32
f16 = mybir.dt.float16

# pair-block offsets: block i = pairs (i, j) for j in i+1..F-1
POFF = [0]
for i in range(F - 1):
    POFF.append(POFF[-1] + (F - 1 - i))
# chunk boundaries on block boundaries, ~4-way balanced: fat ~2 MB
# stores keep the sync ring near peak HBM bandwidth
CHUNKS = [(0, 118), (118, 243), (243, 376), (376, 496)]


def _emit(tc, nc, x_d, w2_d, i128_d, out_d):
    with (
        tc.tile_pool(name="const", bufs=1) as const_pool,
        tc.tile_pool(name="xp", bufs=4) as x_pool,
        tc.tile_pool(name="vidp", bufs=2) as vid_pool,
        tc.tile_pool(name="xtp", bufs=4) as xt_pool,
        tc.tile_pool(name="outp", bufs=6) as out_pool,
        tc.tile_pool(name="ps_t", bufs=2, space="PSUM") as ps_t,
        tc.tile_pool(name="ps_m", bufs=2, space="PSUM") as ps_m,
    ):
        # inputs ride the scalar-engine HWDGE ring, constants first;
        # outputs own the sync HWDGE ring (a shared FIFO would park tile
        # t+1's x load behind tile t's output stores and starve the DVE).
        # constants first (transposes need ident), then tile 0 high
        # quarter (vid group 3 = fields 24-31 is all the first-processed
        # blocks need), then the rest of tile 0, then tiles 1-3 as ONE
        # multi-run DMA (few fat DMAs: the HWDGE FIFO serializes per-DMA
        # fixed latency, so many small loads crawl).
        ident = const_pool.tile([128, 128], f16)
        nc.scalar.dma_start(out=ident[:], in_=i128_d[:])
        w2 = const_pool.tile([128, 128], f16)
        nc.scalar.dma_start(out=w2[:], in_=w2_d[:])
        x0_t = x_pool.tile([128, F * D], f16, tag="xt0")
        Q = F * D // 4  # 512 cols = 8 fields
        nc.scalar.dma_start(
            out=x0_t[:, 3 * Q :].rearrange("p (f d) -> p f d", d=D),
            in_=x_d[0:BT, 24:, :],
        )
        nc.scalar.dma_start(
            out=x0_t[:, : 3 * Q].rearrange("p (f d) -> p f d", d=D),
            in_=x_d[0:BT, :24, :],
        )
        xr_t = x_pool.tile([128, (NTILES - 1) * F * D], f16, tag="xtr")
        nc.scalar.dma_start(
            out=xr_t[:].rearrange("p (t f d) -> p t f d", f=F, d=D),
            in_=x_d[BT:, :, :].rearrange("(t p) f d -> p t f d", p=BT),
        )
        x_ts = [(x0_t, 0)] + [(xr_t, (t - 1) * F * D) for t in range(1, NTILES)]

        for t in range(NTILES):
            b0 = t * BT
            x_t, xc0 = x_ts[t]
            x3 = x_t[:, xc0 : xc0 + F * D].rearrange("p (f d) -> p f d", d=D)

            # vid in 4 descending groups of 4 f-pairs (one PSUM bank each):
            # 4 transposes + 1 ACT copy + 4 matmuls + 1 ACT copy per group.
            # Few fat copies keep the DVE's per-TT wait fan-in low (multi-
            # wait instructions get split into extra event-sem ops on DVE).
            vid_t = vid_pool.tile([128, F * D], f16, tag="vidt")
            for g in reversed(range(4)):
                xT_ps = ps_t.tile([128, 512], f16, tag="xtps")
                for k in range(4):
                    nc.tensor.transpose(
                        xT_ps[:, k * 128 : (k + 1) * 128],
                        x_t[:, xc0 + (4 * g + k) * 128 : xc0 + (4 * g + k + 1) * 128],
                        ident[:],
                    )
                xT_sb = xt_pool.tile([128, 512], f16, tag="xtsb")
                nc.scalar.copy(xT_sb[:], xT_ps[:])
                vid_ps = ps_m.tile([128, 512], f32, tag="vidps")
                for k in range(4):
                    nc.tensor.matmul(
                        vid_ps[:, k * 128 : (k + 1) * 128],
                        xT_sb[:, k * 128 : (k + 1) * 128],
                        w2[:],
                        start=True,
                        stop=True,
                    )
                nc.scalar.copy(vid_t[:, g * 512 : (g + 1) * 512], vid_ps[:])
            vid3 = vid_t[:].rearrange("p (f d) -> p f d", d=D)

            for ci, (c0, c1) in enumerate(reversed(CHUNKS)):
                npair = c1 - c0
                o_t = out_pool.tile([128, npair * D], f16, tag="outs")
                o3 = o_t[:].rearrange("p (q d) -> p q d", d=D)
                for i in reversed(range(F - 1)):
                    blk0, blk1 = POFF[i], POFF[i + 1]
                    lo, hi = max(blk0, c0), min(blk1, c1)
                    if lo >= hi:
                        continue
                    nj = hi - lo
                    j0 = i + 1 + (lo - blk0)
                    nc.vector.tensor_mul(
                        o3[:, lo - c0 : hi - c0, :],
                        x3[:, i : i + 1, :].broadcast_to((128, nj, D)),
                        vid3[:, j0 : j0 + nj, :],
                    )
                if t == 0 and ci == 0:
                    # first chunk streams in pieces so the first store
                    # fires as soon as the first blocks' TTs land
                    subs = ((489, 496), (475, 489), (451, 475), (418, 451), (376, 418))
                elif t == NTILES - 1 and ci == len(CHUNKS) - 1:
                    # last chunk streams in two pieces so the tail store
                    # overlaps the final TTs instead of draining after
                    subs = ((31, 118), (0, 31))
                else:
                    subs = ((c0, c1),)
                # stores alternate between the two HWDGE rings (sync,
                # scalar): per-ring FIFO completion latency stops
                # gating the stream, and the slow-HBM reps get two
                # queues' worth of in-flight packets. Loads are done
                # by the time the first scalar-ring store issues.
                k = t * len(CHUNKS) + ci
                eng = nc.sync if (k % 2 == 0) else nc.scalar
                for s0, s1 in subs:
                    eng.dma_start(
                        out=out_d[b0 : b0 + BT, s0:s1, :],
                        in_=o3[:, s0 - c0 : s1 - c0, :],
                    )


def build_nc():
    nc = bacc.Bacc("TRN2", target_bir_lowering=False, debug=False)
    x_d = nc.dram_tensor("x", [BSH, F, D], f16, kind="ExternalInput")
    w2_d = nc.dram_tensor("W2", [128, 128], f16, kind="ExternalInput")
    i128_d = nc.dram_tensor("I128", [128, 128], f16, kind="ExternalInput")
    out_d = nc.dram_tensor("out", [BSH, P, D], f16, kind="ExternalOutput")
    with tile.TileContext(nc) as tc:
        _emit(tc, nc, x_d.ap(), w2_d.ap(), i128_d.ap(), out_d.ap())
    nc.compile()
    return nc


_NC = None


def kernel(x: np.ndarray, W: np.ndarray, _trace=False, _trace_kwargs=None):
    global _NC
    if _NC is None:
        _NC = build_nc()
    x16 = np.ascontiguousarray(x, dtype=np.float16)
    w2 = np.zeros((128, 128), dtype=np.float16)
    w2[:64, :64] = W.astype(np.float16)
    w2[64:, 64:] = W.astype(np.float16)
    i128 = np.eye(128, dtype=np.float16)
    in_maps = [
        {"x": x16[i * BSH : (i + 1) * BSH], "W2": w2, "I128": i128}
        for i in range(NCORES)
    ]
    res = run_bass_kernel_spmd(
        _NC,
        in_maps,
        core_ids=list(range(NCORES)),
        trace=_trace,
        **(_trace_kwargs or {}),
    )
    out = np.concatenate(
        [res.results[i]["out"].astype(np.float32) for i in range(NCORES)], axis=0
    )
    if _trace:
        return out, res
    return out


# revision 12
# speedup vs baseline: 1.0355x; 1.0355x over previous
"""BilinearInteraction Trainium2 kernel (8 NeuronCores, batch-sharded).

out[b, p=(i,j), d] = x[b, i, d] * (x @ W)[b, j, d]  for the 496 upper-tri
pairs of F=32 fields; x [4096, 32, 64] f32, W [64, 64] f32.

The f32 version of this kernel is pinned at the HBM roofline: 65 MB of
stores per core at the ~358 GB/s HBM-per-NC limit is ~190 us. This
version runs the whole device pipeline in fp16 (intermediate rounding
~1e-3 relative, well inside the 2e-2 gate) and upcasts to f32 on the
host: stores halve to 32.5 MB/core and the DVE multiply runs in the
2x_1P 16-bit perf mode, moving the roofline to ~95-100 us.

Per core: 512 batch rows, processed as 4 tiles of 128 (batch on SBUF
partitions). Per tile, in descending field order so the first-processed
output chunk only needs the tail of vid:
  - vid = x @ W via PE pair-block transposes ([128,128] f-pair blocks
    -> PSUM) + matmuls against a host-provided block-diag [[W,0],[0,W]]
    (two fields per instruction), grouped 4-to-a-PSUM-bank so ACT moves
    PSUM->SBUF in few fat copies (casting f32 PSUM -> fp16 SBUF).
  - pairwise Hadamard on DVE: one tensor_mul per field i covering all
    j>i at once, broadcasting x[:,i,:] over the j axis with a stride-0
    access pattern; innermost dim is 64 contiguous fp16 so the 2x_1P
    packed mode engages.
  - output staged in SBUF in 4 block-aligned chunks (~16 KB/partition
    lines), each DMA'd as one ~2 MB contiguous-per-partition store on
    the sync HWDGE ring; inputs ride the scalar-engine ring so they
    never queue behind output stores.
Ramp: tile 0's x loads high-fields-first (the first-processed chunk
only reads fields >=16) and the first chunk's store is split in three,
so the output stream starts early.
"""

import sys

if "/opt/trn_rl_repo" not in sys.path:
    sys.path.insert(0, "/opt/trn_rl_repo")

import numpy as np

import concourse.bass as bass
import concourse.mybir as mybir
import concourse.tile as tile
from concourse import bacc
from concourse.bass_utils import run_bass_kernel_spmd

B, F, D = 4096, 32, 64
P = F * (F - 1) // 2  # 496
NCORES = 8
BSH = B // NCORES  # 512 batch rows per core
BT = 128  # batch tile (SBUF partitions)
NTILES = BSH // BT  # 4

f32 = mybir.dt.float32
f16 = mybir.dt.float16

# pair-block offsets: block i = pairs (i, j) for j in i+1..F-1
POFF = [0]
for i in range(F - 1):
    POFF.append(POFF[-1] + (F - 1 - i))
# chunk boundaries on block boundaries, ~4-way balanced: fat ~2 MB
# stores keep the sync ring near peak HBM bandwidth
CHUNKS = [(0, 118), (118, 243), (243, 376), (376, 496)]


def _emit(tc, nc, x_d, w2_d, i128_d, out_d):
    with (
        tc.tile_pool(name="const", bufs=1) as const_pool,
        tc.tile_pool(name="xp", bufs=4) as x_pool,
        tc.tile_pool(name="vidp", bufs=2) as vid_pool,
        tc.tile_pool(name="xtp", bufs=4) as xt_pool,
        tc.tile_pool(name="outp", bufs=6) as out_pool,
        tc.tile_pool(name="ps_t", bufs=2, space="PSUM") as ps_t,
        tc.tile_pool(name="ps_m", bufs=2, space="PSUM") as ps_m,
    ):
        # inputs ride the scalar-engine HWDGE ring, constants first;
        # outputs own the sync HWDGE ring (a shared FIFO would park tile
        # t+1's x load behind tile t's output stores and starve the DVE).
        # constants first (transposes need ident), then tile 0 high
        # quarter (vid group 3 = fields 24-31 is all the first-processed
        # blocks need), then the rest of tile 0, then tiles 1-3 as ONE
        # multi-run DMA (few fat DMAs: the HWDGE FIFO serializes per-DMA
        # fixed latency, so many small loads crawl).
        ident = const_pool.tile([128, 128], f16)
        nc.scalar.dma_start(out=ident[:], in_=i128_d[:])
        w2 = const_pool.tile([128, 128], f16)
        nc.scalar.dma_start(out=w2[:], in_=w2_d[:])
        x0_t = x_pool.tile([128, F * D], f16, tag="xt0")
        Q = F * D // 4  # 512 cols = 8 fields
        nc.scalar.dma_start(
            out=x0_t[:, 3 * Q :].rearrange("p (f d) -> p f d", d=D),
            in_=x_d[0:BT, 24:, :],
        )
        nc.scalar.dma_start(
            out=x0_t[:, : 3 * Q].rearrange("p (f d) -> p f d", d=D),
            in_=x_d[0:BT, :24, :],
        )
        xr_t = x_pool.tile([128, (NTILES - 1) * F * D], f16, tag="xtr")
        nc.scalar.dma_start(
            out=xr_t[:].rearrange("p (t f d) -> p t f d", f=F, d=D),
            in_=x_d[BT:, :, :].rearrange("(t p) f d -> p t f d", p=BT),
        )
        x_ts = [(x0_t, 0)] + [(xr_t, (t - 1) * F * D) for t in range(1, NTILES)]

        for t in range(NTILES):
            b0 = t * BT
            x_t, xc0 = x_ts[t]
            x3 = x_t[:, xc0 : xc0 + F * D].rearrange("p (f d) -> p f d", d=D)

            # vid in 4 descending groups of 4 f-pairs (one PSUM bank each):
            # 4 transposes + 1 ACT copy + 4 matmuls + 1 ACT copy per group.
            # Few fat copies keep the DVE's per-TT wait fan-in low (multi-
            # wait instructions get split into extra event-sem ops on DVE).
            vid_t = vid_pool.tile([128, F * D], f16, tag="vidt")
            for g in reversed(range(4)):
                xT_ps = ps_t.tile([128, 512], f16, tag="xtps")
                for k in range(4):
                    nc.tensor.transpose(
                        xT_ps[:, k * 128 : (k + 1) * 128],
                        x_t[:, xc0 + (4 * g + k) * 128 : xc0 + (4 * g + k + 1) * 128],
                        ident[:],
                    )
                xT_sb = xt_pool.tile([128, 512], f16, tag="xtsb")
                nc.scalar.copy(xT_sb[:], xT_ps[:])
                vid_ps = ps_m.tile([128, 512], f32, tag="vidps")
                for k in range(4):
                    nc.tensor.matmul(
                        vid_ps[:, k * 128 : (k + 1) * 128],
                        xT_sb[:, k * 128 : (k + 1) * 128],
                        w2[:],
                        start=True,
                        stop=True,
                    )
                nc.scalar.copy(vid_t[:, g * 512 : (g + 1) * 512], vid_ps[:])
            vid3 = vid_t[:].rearrange("p (f d) -> p f d", d=D)

            for ci, (c0, c1) in enumerate(reversed(CHUNKS)):
                npair = c1 - c0
                o_t = out_pool.tile([128, npair * D], f16, tag="outs")
                o3 = o_t[:].rearrange("p (q d) -> p q d", d=D)
                for i in reversed(range(F - 1)):
                    blk0, blk1 = POFF[i], POFF[i + 1]
                    lo, hi = max(blk0, c0), min(blk1, c1)
                    if lo >= hi:
                        continue
                    nj = hi - lo
                    j0 = i + 1 + (lo - blk0)
                    nc.vector.tensor_mul(
                        o3[:, lo - c0 : hi - c0, :],
                        x3[:, i : i + 1, :].broadcast_to((128, nj, D)),
                        vid3[:, j0 : j0 + nj, :],
                    )
                if t == 0 and ci == 0:
                    # first chunk streams in pieces so the first store
                    # fires as soon as the first blocks' TTs land
                    subs = ((489, 496), (475, 489), (451, 475), (418, 451), (376, 418))
                elif t == NTILES - 1 and ci == len(CHUNKS) - 1:
                    # last chunk streams in two pieces so the tail store
                    # overlaps the final TTs instead of draining after
                    subs = ((31, 118), (0, 31))
                else:
                    subs = ((c0, c1),)
                for s0, s1 in subs:
                    nc.sync.dma_start(
                        out=out_d[b0 : b0 + BT, s0:s1, :],
                        in_=o3[:, s0 - c0 : s1 - c0, :],
                    )


def build_nc():
    nc = bacc.Bacc("TRN2", target_bir_lowering=False, debug=False)
    x_d = nc.dram_tensor("x", [BSH, F, D], f16, kind="ExternalInput")
    w2_d = nc.dram_tensor("W2", [128, 128], f16, kind="ExternalInput")
    i128_d = nc.dram_tensor("I128", [128, 128], f16, kind="ExternalInput")
    out_d = nc.dram_tensor("out", [BSH, P, D], f16, kind="ExternalOutput")
    with tile.TileContext(nc) as tc:
        _emit(tc, nc, x_d.ap(), w2_d.ap(), i128_d.ap(), out_d.ap())
    nc.compile()
    return nc


_NC = None


def kernel(x: np.ndarray, W: np.ndarray, _trace=False, _trace_kwargs=None):
    global _NC
    if _NC is None:
        _NC = build_nc()
    x16 = np.ascontiguousarray(x, dtype=np.float16)
    w2 = np.zeros((128, 128), dtype=np.float16)
    w2[:64, :64] = W.astype(np.float16)
    w2[64:, 64:] = W.astype(np.float16)
    i128 = np.eye(128, dtype=np.float16)
    in_maps = [
        {"x": x16[i * BSH : (i + 1) * BSH], "W2": w2, "I128": i128}
        for i in range(NCORES)
    ]
    res = run_bass_kernel_spmd(
        _NC,
        in_maps,
        core_ids=list(range(NCORES)),
        trace=_trace,
        **(_trace_kwargs or {}),
    )
    out = np.concatenate(
        [res.results[i]["out"].astype(np.float32) for i in range(NCORES)], axis=0
    )
    if _trace:
        return out, res
    return out


# revision 13
# speedup vs baseline: 1.0358x; 1.0003x over previous
"""BilinearInteraction Trainium2 kernel (8 NeuronCores, batch-sharded).

out[b, p=(i,j), d] = x[b, i, d] * (x @ W)[b, j, d]  for the 496 upper-tri
pairs of F=32 fields; x [4096, 32, 64] f32, W [64, 64] f32.

The f32 version of this kernel is pinned at the HBM roofline: 65 MB of
stores per core at the ~358 GB/s HBM-per-NC limit is ~190 us. This
version runs the whole device pipeline in fp16 (intermediate rounding
~1e-3 relative, well inside the 2e-2 gate) and upcasts to f32 on the
host: stores halve to 32.5 MB/core and the DVE multiply runs in the
2x_1P 16-bit perf mode, moving the roofline to ~95-100 us.

Per core: 512 batch rows, processed as 4 tiles of 128 (batch on SBUF
partitions). Per tile, in descending field order so the first-processed
output chunk only needs the tail of vid:
  - vid = x @ W via PE pair-block transposes ([128,128] f-pair blocks
    -> PSUM) + matmuls against a host-provided block-diag [[W,0],[0,W]]
    (two fields per instruction), grouped 4-to-a-PSUM-bank so ACT moves
    PSUM->SBUF in few fat copies (casting f32 PSUM -> fp16 SBUF).
  - pairwise Hadamard on DVE: one tensor_mul per field i covering all
    j>i at once, broadcasting x[:,i,:] over the j axis with a stride-0
    access pattern; innermost dim is 64 contiguous fp16 so the 2x_1P
    packed mode engages.
  - output staged in SBUF in 4 block-aligned chunks (~16 KB/partition
    lines), each DMA'd as one ~2 MB contiguous-per-partition store on
    the sync HWDGE ring; inputs ride the scalar-engine ring so they
    never queue behind output stores.
Ramp: tile 0's x loads high-fields-first (the first-processed chunk
only reads fields >=16) and the first chunk's store is split in three,
so the output stream starts early.
"""

import sys

if "/opt/trn_rl_repo" not in sys.path:
    sys.path.insert(0, "/opt/trn_rl_repo")

import numpy as np

import concourse.bass as bass
import concourse.mybir as mybir
import concourse.tile as tile
from concourse import bacc
from concourse.bass_utils import run_bass_kernel_spmd

B, F, D = 4096, 32, 64
P = F * (F - 1) // 2  # 496
NCORES = 8
BSH = B // NCORES  # 512 batch rows per core
BT = 128  # batch tile (SBUF partitions)
NTILES = BSH // BT  # 4

f32 = mybir.dt.float32
f16 = mybir.dt.float16

# pair-block offsets: block i = pairs (i, j) for j in i+1..F-1
POFF = [0]
for i in range(F - 1):
    POFF.append(POFF[-1] + (F - 1 - i))
# chunk boundaries on block boundaries, ~4-way balanced: fat ~2 MB
# stores keep the sync ring near peak HBM bandwidth
CHUNKS = [(0, 118), (118, 243), (243, 376), (376, 496)]


def _emit(tc, nc, x_d, w2_d, i128_d, out_d):
    with (
        tc.tile_pool(name="const", bufs=1) as const_pool,
        tc.tile_pool(name="xp", bufs=4) as x_pool,
        tc.tile_pool(name="vidp", bufs=2) as vid_pool,
        tc.tile_pool(name="xtp", bufs=4) as xt_pool,
        tc.tile_pool(name="outp", bufs=6) as out_pool,
        tc.tile_pool(name="ps_t", bufs=2, space="PSUM") as ps_t,
        tc.tile_pool(name="ps_m", bufs=2, space="PSUM") as ps_m,
    ):
        # inputs ride the scalar-engine HWDGE ring, constants first;
        # outputs own the sync HWDGE ring (a shared FIFO would park tile
        # t+1's x load behind tile t's output stores and starve the DVE).
        # ramp-critical load order: tile 0 arrives as four per-GROUP
        # loads in descending order (vid group g needs exactly fields
        # 8g..8g+7), with g3 first, then ident/w2.  The PE scheduler
        # interleaves transpose groups ahead of earlier groups' matmuls,
        # so EVERY group's x must land early or it blocks the group-3
        # vid chain (and with it the first TT + first store).
        x0_t = x_pool.tile([128, F * D], f16, tag="xt0")
        Q = F * D // 4  # 512 cols = 8 fields
        nc.scalar.dma_start(
            out=x0_t[:, 3 * Q :].rearrange("p (f d) -> p f d", d=D),
            in_=x_d[0:BT, 24:, :],
        )
        ident = const_pool.tile([128, 128], f16)
        nc.scalar.dma_start(out=ident[:], in_=i128_d[:])
        w2 = const_pool.tile([128, 128], f16)
        nc.scalar.dma_start(out=w2[:], in_=w2_d[:])
        for q in (2, 1, 0):
            nc.scalar.dma_start(
                out=x0_t[:, q * Q : (q + 1) * Q].rearrange(
                    "p (f d) -> p f d", d=D
                ),
                in_=x_d[0:BT, 8 * q : 8 * (q + 1), :],
            )
        xr_t = x_pool.tile([128, (NTILES - 1) * F * D], f16, tag="xtr")
        nc.scalar.dma_start(
            out=xr_t[:].rearrange("p (t f d) -> p t f d", f=F, d=D),
            in_=x_d[BT:, :, :].rearrange("(t p) f d -> p t f d", p=BT),
        )
        x_ts = [(x0_t, 0)] + [(xr_t, (t - 1) * F * D) for t in range(1, NTILES)]

        for t in range(NTILES):
            b0 = t * BT
            x_t, xc0 = x_ts[t]
            x3 = x_t[:, xc0 : xc0 + F * D].rearrange("p (f d) -> p f d", d=D)

            # vid in 4 descending groups of 4 f-pairs (one PSUM bank each):
            # 4 transposes + 1 ACT copy + 4 matmuls + 1 ACT copy per group.
            # Few fat copies keep the DVE's per-TT wait fan-in low (multi-
            # wait instructions get split into extra event-sem ops on DVE).
            vid_t = vid_pool.tile([128, F * D], f16, tag="vidt")
            for g in reversed(range(4)):
                xT_ps = ps_t.tile([128, 512], f16, tag="xtps")
                for k in range(4):
                    nc.tensor.transpose(
                        xT_ps[:, k * 128 : (k + 1) * 128],
                        x_t[:, xc0 + (4 * g + k) * 128 : xc0 + (4 * g + k + 1) * 128],
                        ident[:],
                    )
                xT_sb = xt_pool.tile([128, 512], f16, tag="xtsb")
                nc.scalar.copy(xT_sb[:], xT_ps[:])
                vid_ps = ps_m.tile([128, 512], f32, tag="vidps")
                for k in range(4):
                    nc.tensor.matmul(
                        vid_ps[:, k * 128 : (k + 1) * 128],
                        xT_sb[:, k * 128 : (k + 1) * 128],
                        w2[:],
                        start=True,
                        stop=True,
                    )
                nc.scalar.copy(vid_t[:, g * 512 : (g + 1) * 512], vid_ps[:])
            vid3 = vid_t[:].rearrange("p (f d) -> p f d", d=D)

            for ci, (c0, c1) in enumerate(reversed(CHUNKS)):
                npair = c1 - c0
                o_t = out_pool.tile([128, npair * D], f16, tag="outs")
                o3 = o_t[:].rearrange("p (q d) -> p q d", d=D)
                for i in reversed(range(F - 1)):
                    blk0, blk1 = POFF[i], POFF[i + 1]
                    lo, hi = max(blk0, c0), min(blk1, c1)
                    if lo >= hi:
                        continue
                    nj = hi - lo
                    j0 = i + 1 + (lo - blk0)
                    nc.vector.tensor_mul(
                        o3[:, lo - c0 : hi - c0, :],
                        x3[:, i : i + 1, :].broadcast_to((128, nj, D)),
                        vid3[:, j0 : j0 + nj, :],
                    )
                if t == 0 and ci == 0:
                    # first chunk streams in pieces so the first store
                    # fires as soon as the first blocks' TTs land
                    subs = ((489, 496), (475, 489), (451, 475), (418, 451), (376, 418))
                elif t == NTILES - 1 and ci == len(CHUNKS) - 1:
                    # last chunk streams in two pieces so the tail store
                    # overlaps the final TTs instead of draining after
                    subs = ((31, 118), (0, 31))
                else:
                    subs = ((c0, c1),)
                for s0, s1 in subs:
                    nc.sync.dma_start(
                        out=out_d[b0 : b0 + BT, s0:s1, :],
                        in_=o3[:, s0 - c0 : s1 - c0, :],
                    )


def build_nc():
    nc = bacc.Bacc("TRN2", target_bir_lowering=False, debug=False)
    x_d = nc.dram_tensor("x", [BSH, F, D], f16, kind="ExternalInput")
    w2_d = nc.dram_tensor("W2", [128, 128], f16, kind="ExternalInput")
    i128_d = nc.dram_tensor("I128", [128, 128], f16, kind="ExternalInput")
    out_d = nc.dram_tensor("out", [BSH, P, D], f16, kind="ExternalOutput")
    with tile.TileContext(nc) as tc:
        _emit(tc, nc, x_d.ap(), w2_d.ap(), i128_d.ap(), out_d.ap())
    nc.compile()
    return nc


_NC = None


def kernel(x: np.ndarray, W: np.ndarray, _trace=False, _trace_kwargs=None):
    global _NC
    if _NC is None:
        _NC = build_nc()
    x16 = np.ascontiguousarray(x, dtype=np.float16)
    w2 = np.zeros((128, 128), dtype=np.float16)
    w2[:64, :64] = W.astype(np.float16)
    w2[64:, 64:] = W.astype(np.float16)
    i128 = np.eye(128, dtype=np.float16)
    in_maps = [
        {"x": x16[i * BSH : (i + 1) * BSH], "W2": w2, "I128": i128}
        for i in range(NCORES)
    ]
    res = run_bass_kernel_spmd(
        _NC,
        in_maps,
        core_ids=list(range(NCORES)),
        trace=_trace,
        **(_trace_kwargs or {}),
    )
    out = np.concatenate(
        [res.results[i]["out"].astype(np.float32) for i in range(NCORES)], axis=0
    )
    if _trace:
        return out, res
    return out
